# revision 5
# baseline (speedup 1.0000x reference)
"""Per-class variance penalty (segment-reduce) on 8 TRN2 NeuronCores.

Strategy (data-parallel over N): each core streams its 1/8 shard of x
through the TensorEngine as ``stats += onehot(t)^T @ [x | x^2]``,
accumulating per-class sums and sums-of-squares for all 100 classes in a
single PSUM bank across 256 row-tiles.  The one-hot is built on-chip by
comparing an iota row against the class id (per-partition scalar).  The
8 partial [C, 2D] statistics are summed on the host, where the final
(tiny) [C, D] variance/L1 reduction runs in numpy.

Data is shipped as bf16 (exactly representable one-hot weights, bf16
x / x^2 streams, fp32 PSUM accumulation).  The output is a single scalar
averaged over C*D = 25.6k statistics, so the bf16 rounding noise averages
out ~1e-4 relative, while halving HBM traffic.

Per-engine layout (tuned against the NTFF profile):
  - DMA is issued in 16-tile groups (1 MiB per dma_start) to amortize the
    ~625 ns HWDGE issue cost that dominated the naive version.
  - Each group tile is [128, G, 2, D]: half 0 is DMA'd x, half 1 is x^2,
    so each row-tile's matmul reads 512 contiguous columns.
  - Squares are computed on ScalarE (ACT) for part of the group and
    VectorE (DVE) for the rest; one-hots alternate DVE / GpSimd.
"""

import numpy as np
import ml_dtypes

import concourse.bass as bass
import concourse.tile as tile
from concourse import bacc, mybir
from concourse.bass_utils import run_bass_kernel_spmd

N_CORES = 8
N, D, C = 262144, 256, 100
N_SHARD = N // N_CORES          # 32768 rows per core
P = 128                          # SBUF partitions / PE contraction dim
N_TILES = N_SHARD // P           # 256 row-tiles per core
G = 16                           # row-tiles per DMA group
N_GROUPS = N_TILES // G
A_ACT = 10                       # tiles per group squared on ScalarE (rest DVE)
BF16 = mybir.dt.bfloat16
FP32 = mybir.dt.float32

_compiled = None


def _build():
    nc = bacc.Bacc("TRN2", target_bir_lowering=False, debug=False,
                   num_devices=N_CORES)
    x_d = nc.dram_tensor("x", [N_SHARD, D], BF16, kind="ExternalInput").ap()
    t_d = nc.dram_tensor("t", [P, N_TILES], FP32, kind="ExternalInput").ap()
    iota_d = nc.dram_tensor("iota", [P, P], BF16, kind="ExternalInput").ap()
    stats_d = nc.dram_tensor("stats", [P, 2 * D], FP32,
                             kind="ExternalOutput").ap()

    with tile.TileContext(nc) as tc:
        with (
            tc.tile_pool(name="const", bufs=1) as const_pool,
            tc.tile_pool(name="xg", bufs=3) as x_pool,
            tc.tile_pool(name="oh", bufs=2 * G) as oh_pool,
            tc.tile_pool(name="psum", bufs=1, space=bass.MemorySpace.PSUM) as psum_pool,
        ):
            tsb = const_pool.tile([P, N_TILES], FP32, tag="tsb")
            nc.sync.dma_start(tsb[:], t_d[:])
            iota = const_pool.tile([P, P], BF16, tag="iota")
            nc.sync.dma_start(iota[:], iota_d[:])

            acc = psum_pool.tile([P, 2 * D], FP32)

            for g in range(N_GROUPS):
                xt = x_pool.tile([P, G * 2 * D], BF16)
                xv = xt[:].rearrange("p (j h d) -> p j h d", j=G, h=2, d=D)
                src = x_d[g * G * P:(g + 1) * G * P, :].rearrange(
                    "(j p) d -> p j d", p=P)
                nc.sync.dma_start(xv[:, :, 0, :], src)

                # squares: ACT takes the first A_ACT tiles, DVE the rest
                nc.scalar.activation(xv[:, 0:A_ACT, 1, :], xv[:, 0:A_ACT, 0, :],
                                     mybir.ActivationFunctionType.Square)
                nc.vector.tensor_mul(xv[:, A_ACT:, 1, :], xv[:, A_ACT:, 0, :],
                                     xv[:, A_ACT:, 0, :])

                for j in range(G):
                    i = g * G + j
                    oh = oh_pool.tile([P, P], BF16)
                    eng = nc.vector if (j % 2 == 0) else nc.gpsimd
                    eng.tensor_scalar(oh[:], iota[:], tsb[:, i:i + 1], None,
                                      mybir.AluOpType.is_equal)
                    nc.tensor.matmul(acc[:], oh[:], xv[:, j, :, :],
                                     start=(i == 0), stop=(i == N_TILES - 1))

            out_sb = const_pool.tile([P, 2 * D], FP32, tag="out_sb")
            nc.vector.tensor_copy(out_sb[:], acc[:])
            nc.sync.dma_start(stats_d[:], out_sb[:])

    nc.compile()
    return nc


def _prepare_in_maps(x: np.ndarray, t: np.ndarray) -> list[dict]:
    xh = np.asarray(x).astype(ml_dtypes.bfloat16)
    t = np.asarray(t)
    iota = np.broadcast_to(np.arange(P, dtype=np.float32), (P, P)).astype(
        ml_dtypes.bfloat16)
    in_maps = []
    for c in range(N_CORES):
        xs = xh[c * N_SHARD:(c + 1) * N_SHARD]
        ts = t[c * N_SHARD:(c + 1) * N_SHARD]
        # tsb[p, i] = class id of row i*P + p of this shard
        tsb = np.ascontiguousarray(
            ts.reshape(N_TILES, P).T.astype(np.float32))
        in_maps.append({"x": xs, "t": tsb, "iota": iota})
    return in_maps


def kernel(x: np.ndarray, t: np.ndarray) -> np.ndarray:
    global _compiled
    if _compiled is None:
        _compiled = _build()
    nc = _compiled

    t = np.asarray(t)
    in_maps = _prepare_in_maps(x, t)
    res = run_bass_kernel_spmd(nc, in_maps, list(range(N_CORES)))

    s = np.zeros((C, D), np.float32)
    sq = np.zeros((C, D), np.float32)
    for c in range(N_CORES):
        stats = res.results[c]["stats"]
        s += stats[:C, 0:D]
        sq += stats[:C, D:2 * D]

    cnt = np.bincount(t.astype(np.int64), minlength=C).astype(np.float32)
    n = cnt[:, None]
    var = (sq - s * s / n) / (n - 1.0)
    penalty = np.abs(var).sum(dtype=np.float32) / np.float32(C)
    return np.asarray(penalty, dtype=np.float32).reshape(1)


# revision 6
# speedup vs baseline: 2.7164x; 2.7164x over previous
"""Per-class variance penalty (segment-reduce) on 8 TRN2 NeuronCores.

Strategy (data-parallel over N): each core streams its 1/8 shard of x
through the TensorEngine as ``stats += onehot(t)^T @ [x | x^2]``,
accumulating per-class sums and sums-of-squares for all 100 classes in a
single PSUM bank across 256 row-tiles.  The one-hot is built on-chip by
comparing an iota row against the class id (per-partition scalar).  The
8 partial [C, 2D] statistics are summed on the host, where the final
(tiny) [C, D] variance/L1 reduction runs in numpy.

Data is shipped as bf16 (exactly representable one-hot weights, bf16
x / x^2 streams, fp32 PSUM accumulation).  The output is a single scalar
averaged over C*D = 25.6k statistics, so the bf16 rounding noise averages
out ~1e-4 relative, while halving HBM traffic.

Per-engine layout (tuned against the NTFF profile):
  - DMA is issued in 16-tile groups (1 MiB per dma_start) to amortize the
    ~625 ns HWDGE issue cost that dominated the naive version.
  - Each group tile is [128, G, 2, D]: half 0 is DMA'd x, half 1 is x^2,
    so each row-tile's matmul reads 512 contiguous columns.
  - Squares are computed on ScalarE (ACT) for part of the group and
    VectorE (DVE) for the rest; one-hots alternate DVE / GpSimd.
"""

import numpy as np
import ml_dtypes

import concourse.bass as bass
import concourse.tile as tile
from concourse import bacc, mybir
from concourse.bass_utils import run_bass_kernel_spmd

N_CORES = 8
N, D, C = 262144, 256, 100
N_SHARD = N // N_CORES          # 32768 rows per core
P = 128                          # SBUF partitions / PE contraction dim
N_TILES = N_SHARD // P           # 256 row-tiles per core
G = 16                           # row-tiles per DMA group
N_GROUPS = N_TILES // G
A_ACT = 10                       # tiles per group squared on ScalarE (rest DVE)
BF16 = mybir.dt.bfloat16
FP32 = mybir.dt.float32

_compiled = None


def _build():
    nc = bacc.Bacc("TRN2", target_bir_lowering=False, debug=False,
                   num_devices=N_CORES)
    x_d = nc.dram_tensor("x", [N_SHARD, D], BF16, kind="ExternalInput").ap()
    t_d = nc.dram_tensor("t", [P, N_TILES], FP32, kind="ExternalInput").ap()
    iota_d = nc.dram_tensor("iota", [P, P], BF16, kind="ExternalInput").ap()
    stats_d = nc.dram_tensor("stats", [P, 2 * D], FP32,
                             kind="ExternalOutput").ap()

    with tile.TileContext(nc) as tc:
        with (
            tc.tile_pool(name="const", bufs=1) as const_pool,
            tc.tile_pool(name="xg", bufs=3) as x_pool,
            tc.tile_pool(name="oh", bufs=2 * G) as oh_pool,
            tc.tile_pool(name="psum", bufs=1, space=bass.MemorySpace.PSUM) as psum_pool,
        ):
            tsb = const_pool.tile([P, N_TILES], FP32, tag="tsb")
            nc.sync.dma_start(tsb[:], t_d[:])
            iota = const_pool.tile([P, P], BF16, tag="iota")
            nc.sync.dma_start(iota[:], iota_d[:])

            acc = psum_pool.tile([P, 2 * D], FP32)

            for g in range(N_GROUPS):
                xt = x_pool.tile([P, G * 2 * D], BF16)
                xv = xt[:].rearrange("p (j h d) -> p j h d", j=G, h=2, d=D)
                src = x_d[g * G * P:(g + 1) * G * P, :].rearrange(
                    "(j p) d -> p j d", p=P)
                nc.sync.dma_start(xv[:, :, 0, :], src)

                # squares all on ScalarE (DVE is saturated by one-hots)
                nc.scalar.activation(xv[:, :, 1, :], xv[:, :, 0, :],
                                     mybir.ActivationFunctionType.Square)

                for j in range(G):
                    i = g * G + j
                    oh = oh_pool.tile([P, P], BF16)
                    nc.vector.tensor_scalar(oh[:], iota[:], tsb[:, i:i + 1],
                                            None, mybir.AluOpType.is_equal)
                    nc.tensor.matmul(acc[:], oh[:], xv[:, j, :, :],
                                     start=(i == 0), stop=(i == N_TILES - 1))

            out_sb = const_pool.tile([P, 2 * D], FP32, tag="out_sb")
            nc.vector.tensor_copy(out_sb[:], acc[:])
            nc.sync.dma_start(stats_d[:], out_sb[:])

    nc.compile()
    return nc


def _prepare_in_maps(x: np.ndarray, t: np.ndarray) -> list[dict]:
    xh = np.asarray(x).astype(ml_dtypes.bfloat16)
    t = np.asarray(t)
    iota = np.broadcast_to(np.arange(P, dtype=np.float32), (P, P)).astype(
        ml_dtypes.bfloat16)
    in_maps = []
    for c in range(N_CORES):
        xs = xh[c * N_SHARD:(c + 1) * N_SHARD]
        ts = t[c * N_SHARD:(c + 1) * N_SHARD]
        # tsb[p, i] = class id of row i*P + p of this shard
        tsb = np.ascontiguousarray(
            ts.reshape(N_TILES, P).T.astype(np.float32))
        in_maps.append({"x": xs, "t": tsb, "iota": iota})
    return in_maps


def kernel(x: np.ndarray, t: np.ndarray) -> np.ndarray:
    global _compiled
    if _compiled is None:
        _compiled = _build()
    nc = _compiled

    t = np.asarray(t)
    in_maps = _prepare_in_maps(x, t)
    res = run_bass_kernel_spmd(nc, in_maps, list(range(N_CORES)))

    s = np.zeros((C, D), np.float32)
    sq = np.zeros((C, D), np.float32)
    for c in range(N_CORES):
        stats = res.results[c]["stats"]
        s += stats[:C, 0:D]
        sq += stats[:C, D:2 * D]

    cnt = np.bincount(t.astype(np.int64), minlength=C).astype(np.float32)
    n = cnt[:, None]
    var = (sq - s * s / n) / (n - 1.0)
    penalty = np.abs(var).sum(dtype=np.float32) / np.float32(C)
    return np.asarray(penalty, dtype=np.float32).reshape(1)


# revision 8
# speedup vs baseline: 3.1635x; 1.1646x over previous
"""Per-class variance penalty (segment-reduce) on 8 TRN2 NeuronCores.

Strategy (data-parallel over N): each core streams its 1/8 shard of x
through the TensorEngine as ``stats += onehot(t)^T @ [x | x^2]``,
accumulating per-class sums and sums-of-squares for all 100 classes in a
single PSUM bank across 256 row-tiles.  The one-hot is built on-chip by
comparing an iota row against the class id (per-partition scalar).  The
8 partial [C, 2D] statistics are summed on the host, where the final
(tiny) [C, D] variance/L1 reduction runs in numpy.

Data is shipped as bf16 (exactly representable one-hot weights, bf16
x / x^2 streams, fp32 PSUM accumulation).  The output is a single scalar
averaged over C*D = 25.6k statistics, so the bf16 rounding noise averages
out ~1e-4 relative, while halving HBM traffic.

Per-engine layout (tuned against the NTFF profile):
  - DMA is issued in 16-tile groups (1 MiB per dma_start) to amortize the
    ~625 ns HWDGE issue cost that dominated the naive version.
  - Each group tile is [128, G, 2, D]: half 0 is DMA'd x, half 1 is x^2,
    so each row-tile's matmul reads 512 contiguous columns.
  - Squares are computed on ScalarE (ACT) for part of the group and
    VectorE (DVE) for the rest; one-hots alternate DVE / GpSimd.
"""

import numpy as np
import ml_dtypes

import concourse.bass as bass
import concourse.tile as tile
from concourse import bacc, mybir
from concourse.bass_utils import run_bass_kernel_spmd

N_CORES = 8
N, D, C = 262144, 256, 100
N_SHARD = N // N_CORES          # 32768 rows per core
P = 128                          # SBUF partitions / PE contraction dim
N_TILES = N_SHARD // P           # 256 row-tiles per core
G = 16                           # row-tiles per DMA group
N_GROUPS = N_TILES // G
A_ACT = 10                       # tiles per group squared on ScalarE (rest DVE)
BF16 = mybir.dt.bfloat16
FP32 = mybir.dt.float32

_compiled = None


def _build():
    nc = bacc.Bacc("TRN2", target_bir_lowering=False, debug=False,
                   num_devices=N_CORES)
    x_d = nc.dram_tensor("x", [N_SHARD, D], BF16, kind="ExternalInput").ap()
    t_d = nc.dram_tensor("t", [P, N_TILES], FP32, kind="ExternalInput").ap()
    iota_d = nc.dram_tensor("iota", [P, P], BF16, kind="ExternalInput").ap()
    stats_d = nc.dram_tensor("stats", [P, 2 * D], FP32,
                             kind="ExternalOutput").ap()

    with tile.TileContext(nc) as tc:
        with (
            tc.tile_pool(name="const", bufs=1) as const_pool,
            tc.tile_pool(name="xg", bufs=4) as x_pool,
            tc.tile_pool(name="oh", bufs=2 * G) as oh_pool,
            tc.tile_pool(name="psum", bufs=1, space=bass.MemorySpace.PSUM) as psum_pool,
        ):
            tsb = const_pool.tile([P, N_TILES], FP32, tag="tsb")
            nc.sync.dma_start(tsb[:], t_d[:])
            iota = const_pool.tile([P, P], BF16, tag="iota")
            nc.sync.dma_start(iota[:], iota_d[:])

            acc = psum_pool.tile([P, 2 * D], FP32)

            for g in range(N_GROUPS):
                xt = x_pool.tile([P, G * 2 * D], BF16)
                xv = xt[:].rearrange("p (j h d) -> p j h d", j=G, h=2, d=D)
                src = x_d[g * G * P:(g + 1) * G * P, :].rearrange(
                    "(j p) d -> p j d", p=P)
                nc.sync.dma_start(xv[:, :, 0, :], src)

                # squares: 12 tiles on ScalarE (two ops, so the first
                # tiles' matmuls unblock sooner), 4 on VectorE
                nc.scalar.activation(xv[:, 0:6, 1, :], xv[:, 0:6, 0, :],
                                     mybir.ActivationFunctionType.Square)
                nc.scalar.activation(xv[:, 6:12, 1, :], xv[:, 6:12, 0, :],
                                     mybir.ActivationFunctionType.Square)
                nc.vector.tensor_mul(xv[:, 12:, 1, :], xv[:, 12:, 0, :],
                                     xv[:, 12:, 0, :])

                for j in range(G):
                    i = g * G + j
                    oh = oh_pool.tile([P, P], BF16)
                    nc.vector.tensor_scalar(oh[:], iota[:], tsb[:, i:i + 1],
                                            None, mybir.AluOpType.is_equal)
                    nc.tensor.matmul(acc[:], oh[:], xv[:, j, :, :],
                                     start=(i == 0), stop=(i == N_TILES - 1))

            out_sb = const_pool.tile([P, 2 * D], FP32, tag="out_sb")
            nc.vector.tensor_copy(out_sb[:], acc[:])
            nc.sync.dma_start(stats_d[:], out_sb[:])

    nc.compile()
    return nc


def _prepare_in_maps(x: np.ndarray, t: np.ndarray) -> list[dict]:
    xh = np.asarray(x).astype(ml_dtypes.bfloat16)
    t = np.asarray(t)
    iota = np.broadcast_to(np.arange(P, dtype=np.float32), (P, P)).astype(
        ml_dtypes.bfloat16)
    in_maps = []
    for c in range(N_CORES):
        xs = xh[c * N_SHARD:(c + 1) * N_SHARD]
        ts = t[c * N_SHARD:(c + 1) * N_SHARD]
        # tsb[p, i] = class id of row i*P + p of this shard
        tsb = np.ascontiguousarray(
            ts.reshape(N_TILES, P).T.astype(np.float32))
        in_maps.append({"x": xs, "t": tsb, "iota": iota})
    return in_maps


def kernel(x: np.ndarray, t: np.ndarray) -> np.ndarray:
    global _compiled
    if _compiled is None:
        _compiled = _build()
    nc = _compiled

    t = np.asarray(t)
    in_maps = _prepare_in_maps(x, t)
    res = run_bass_kernel_spmd(nc, in_maps, list(range(N_CORES)))

    s = np.zeros((C, D), np.float32)
    sq = np.zeros((C, D), np.float32)
    for c in range(N_CORES):
        stats = res.results[c]["stats"]
        s += stats[:C, 0:D]
        sq += stats[:C, D:2 * D]

    cnt = np.bincount(t.astype(np.int64), minlength=C).astype(np.float32)
    n = cnt[:, None]
    var = (sq - s * s / n) / (n - 1.0)
    penalty = np.abs(var).sum(dtype=np.float32) / np.float32(C)
    return np.asarray(penalty, dtype=np.float32).reshape(1)


# revision 11
# speedup vs baseline: 3.6570x; 1.1560x over previous
"""Per-class variance penalty (segment-reduce) on 8 TRN2 NeuronCores.

Strategy (data-parallel over N): each core streams its 1/8 shard of x
through the TensorEngine as ``stats += onehot(t)^T @ [x | x^2]``,
accumulating per-class sums and sums-of-squares for all 100 classes in a
single PSUM bank across 256 row-tiles.  The one-hot is built on-chip by
comparing an iota row against the class id (per-partition scalar).  The
8 partial [C, 2D] statistics are summed on the host, where the final
(tiny) [C, D] variance/L1 reduction runs in numpy.

Data is shipped as bf16 (exactly representable one-hot weights, bf16
x / x^2 streams, fp32 PSUM accumulation).  The output is a single scalar
averaged over C*D = 25.6k statistics, so the bf16 rounding noise averages
out ~1e-4 relative, while halving HBM traffic.

Per-engine layout (tuned against the NTFF profile):
  - DMA is issued in 16-tile groups (1 MiB per dma_start) to amortize the
    ~625 ns HWDGE issue cost that dominated the naive version.
  - Each group tile is [128, G, 2, D]: half 0 is DMA'd x, half 1 is x^2,
    so each row-tile's matmul reads 512 contiguous columns.
  - Squares are computed on ScalarE (ACT) for part of the group and
    VectorE (DVE) for the rest; one-hots alternate DVE / GpSimd.
"""

import numpy as np
import ml_dtypes

import concourse.bass as bass
import concourse.tile as tile
from concourse import bacc, mybir
from concourse.bass_utils import run_bass_kernel_spmd

N_CORES = 8
N, D, C = 262144, 256, 100
N_SHARD = N // N_CORES          # 32768 rows per core
P = 128                          # SBUF partitions / PE contraction dim
N_TILES = N_SHARD // P           # 256 row-tiles per core
G = 16                           # row-tiles per DMA group
N_GROUPS = N_TILES // G
A_ACT = 10                       # tiles per group squared on ScalarE (rest DVE)
BF16 = mybir.dt.bfloat16
FP32 = mybir.dt.float32

_compiled = None


def _build():
    nc = bacc.Bacc("TRN2", target_bir_lowering=False, debug=False,
                   num_devices=N_CORES)
    # x is shipped host-reordered as [group*P, G*D]: row g*P+p holds the
    # G row-tiles' data of partition p (8 KiB contiguous per partition per
    # group) so each DMA uses fat descriptors instead of 512 B ones.
    x_d = nc.dram_tensor("x", [N_GROUPS * P, G * D], BF16,
                         kind="ExternalInput").ap()
    t_d = nc.dram_tensor("t", [P, N_TILES], FP32, kind="ExternalInput").ap()
    iota_d = nc.dram_tensor("iota", [P, P], BF16, kind="ExternalInput").ap()
    stats_d = nc.dram_tensor("stats", [P, 2 * D], FP32,
                             kind="ExternalOutput").ap()

    with tile.TileContext(nc) as tc:
        with (
            tc.tile_pool(name="const", bufs=1) as const_pool,
            tc.tile_pool(name="xg", bufs=4) as x_pool,
            tc.tile_pool(name="oh", bufs=2 * G) as oh_pool,
            tc.tile_pool(name="psum", bufs=1, space=bass.MemorySpace.PSUM) as psum_pool,
        ):
            tsb = const_pool.tile([P, N_TILES], FP32, tag="tsb")
            nc.sync.dma_start(tsb[:], t_d[:])
            iota = const_pool.tile([P, P], BF16, tag="iota")
            nc.sync.dma_start(iota[:], iota_d[:])

            acc = psum_pool.tile([P, 2 * D], FP32)

            for g in range(N_GROUPS):
                # tile layout [p, half, j, d]: half 0 = x (one contiguous
                # 8 KiB DMA per partition), half 1 = x^2.  The matmul rhs
                # for row-tile j is the strided AP (half, d) -> 512 cols.
                xt = x_pool.tile([P, 2 * G * D], BF16)
                xv = xt[:].rearrange("p (h j d) -> p h j d", h=2, j=G, d=D)
                nc.sync.dma_start(xv[:, 0, :, :], x_d[g * P:(g + 1) * P, :])

                # squares: 12 tiles on ScalarE (two ops, so the first
                # tiles' matmuls unblock sooner), 4 on VectorE
                nc.scalar.activation(xv[:, 1, 0:6, :], xv[:, 0, 0:6, :],
                                     mybir.ActivationFunctionType.Square)
                nc.scalar.activation(xv[:, 1, 6:12, :], xv[:, 0, 6:12, :],
                                     mybir.ActivationFunctionType.Square)
                nc.vector.tensor_mul(xv[:, 1, 12:, :], xv[:, 0, 12:, :],
                                     xv[:, 0, 12:, :])

                for j in range(G):
                    i = g * G + j
                    oh = oh_pool.tile([P, P], BF16)
                    nc.vector.tensor_scalar(oh[:], iota[:], tsb[:, i:i + 1],
                                            None, mybir.AluOpType.is_equal)
                    nc.tensor.matmul(acc[:], oh[:], xv[:, :, j, :],
                                     start=(i == 0), stop=(i == N_TILES - 1))

            out_sb = const_pool.tile([P, 2 * D], FP32, tag="out_sb")
            nc.vector.tensor_copy(out_sb[:], acc[:])
            nc.sync.dma_start(stats_d[:], out_sb[:])

    nc.compile()
    return nc


def _prepare_in_maps(x: np.ndarray, t: np.ndarray) -> list[dict]:
    xh = np.asarray(x).astype(ml_dtypes.bfloat16)
    t = np.asarray(t)
    iota = np.broadcast_to(np.arange(P, dtype=np.float32), (P, P)).astype(
        ml_dtypes.bfloat16)
    in_maps = []
    for c in range(N_CORES):
        xs = xh[c * N_SHARD:(c + 1) * N_SHARD]
        # regroup to [g, p, j, d] so each (g, p) block is contiguous
        xs = np.ascontiguousarray(
            xs.reshape(N_GROUPS, G, P, D).transpose(0, 2, 1, 3)
        ).reshape(N_GROUPS * P, G * D)
        ts = t[c * N_SHARD:(c + 1) * N_SHARD]
        # tsb[p, i] = class id of row i*P + p of this shard
        tsb = np.ascontiguousarray(
            ts.reshape(N_TILES, P).T.astype(np.float32))
        in_maps.append({"x": xs, "t": tsb, "iota": iota})
    return in_maps


def kernel(x: np.ndarray, t: np.ndarray) -> np.ndarray:
    global _compiled
    if _compiled is None:
        _compiled = _build()
    nc = _compiled

    t = np.asarray(t)
    in_maps = _prepare_in_maps(x, t)
    res = run_bass_kernel_spmd(nc, in_maps, list(range(N_CORES)))

    s = np.zeros((C, D), np.float32)
    sq = np.zeros((C, D), np.float32)
    for c in range(N_CORES):
        stats = res.results[c]["stats"]
        s += stats[:C, 0:D]
        sq += stats[:C, D:2 * D]

    cnt = np.bincount(t.astype(np.int64), minlength=C).astype(np.float32)
    n = cnt[:, None]
    var = (sq - s * s / n) / (n - 1.0)
    penalty = np.abs(var).sum(dtype=np.float32) / np.float32(C)
    return np.asarray(penalty, dtype=np.float32).reshape(1)
